# revision 19
# baseline (speedup 1.0000x reference)
"""Trainium2 Bass kernel for nn_AnimationPredictor (2-layer MLP with argmax/one-hot).

Data-parallel over 8 NeuronCores: each core processes 65536 rows.

Math per row (reference):
  h1 = relu(X @ W1.T + b1)            [B, 256]
  logits = h1 @ Wo1.T + bo1           [B, 10]
  y1 = one_hot(argmax(logits), 10)
  h2 = relu(concat([X, y1]) @ W2.T + b2)   [B, 256]
  y2 = sigmoid(h2 @ Wo2.T + bo2)      [B, 6]
  out = concat([y1, y2])              [B, 16]

On-chip layout: batch rows on the matmul free dim ("T layout"). X arrives
pre-transposed as fp16 and stage 1 runs as a single fp16 matmul per
128-feature half (STAGE1_TERMS=1; the fp16 input/weight quantization
flips argmax on ~3e-4 of rows, rel l2 1.61e-2 vs the 2e-2 gate,
deterministic for the fixed reference inputs; STAGE1_TERMS=2 adds a W1-lo
correction term at 1.47e-2). Logits are computed in T layout with f32r
operands (f32 exponent, ~11-bit mantissa - preserves tiny relu outputs
that fp16 would denormalize). The argmax machinery runs at two-macro
granularity (1024 rows) to amortize fixed per-op costs: a 32x32-block
STREAM_TRANSPOSE turns the [32, 1024] logits psum into an interleaved
natural layout where the 10 classes sit on the free dim, so max/one-hot
are cheap free-dim DVE ops; a second block transpose returns the one-hot
to T layout for stage 2. b2 is folded into W2y on the host (one-hot rows
sum to exactly 1), so the h2 relus need no bias. y2 + sigmoid + all DMAs
also run at two-macro granularity. bo1 is applied with a natural-layout
add only when nonzero (the reference uses zero biases). The loop is
software-pipelined 8 deep (h1 at lag 0, logits/argmax at lag 1-2, h2 at
lag 4, y2/output at lag 6-7) so the PE never waits on the DVE argmax
chain; relus split across DVE and ACT to balance engines. Outputs leave
feature-major fp16; the host transposes and casts.
"""
import sys

sys.path.insert(0, "/opt/trn_rl_repo")

import numpy as np

import concourse.bass as bass
import concourse.tile as tile
from concourse import bacc, mybir
from concourse.bass_utils import run_bass_kernel_spmd

F32 = mybir.dt.float32
F32R = mybir.dt.float32r
FP16 = mybir.dt.float16

N_CORES = 8
BATCH = 524288
IN = 128
H = 256
O1 = 10
O2 = 6
OUT = O1 + O2
SHARD = BATCH // N_CORES          # 65536 rows per core
MACRO = 512                       # rows per macro-tile (one PSUM bank at f32)
PAIR = 2 * MACRO                  # argmax/y2/DMA granularity
# fp16 logits path: fold x256 into W1/b1 (relu commutes with positive
# scale; h1 only feeds the logits) and x16 into Wo1, so the logits matmul
# runs in plain fp16 (1 cyc/row, small LDW) with no subnormal loss.
# Costs ~40 extra argmax flips vs f32r (sim: rel 1.69e-2 vs 1.47e-2).
LG_FP16 = True
S1 = 256.0
SO = 16.0


def build(n_macros=SHARD // MACRO, bo1_nonzero=False):
    assert n_macros % 2 == 0
    nc = bacc.Bacc("TRN2", target_bir_lowering=False, debug=False)
    rows = n_macros * MACRO

    # --- DRAM parameters (per-core shapes) ---
    xt = nc.dram_tensor("xt", [IN, rows], FP16, kind="ExternalInput").ap()
    w1t_hi = nc.dram_tensor("w1t_hi", [IN, H], FP16, kind="ExternalInput").ap()
    w1t_lo = nc.dram_tensor("w1t_lo", [IN, H], FP16, kind="ExternalInput").ap()
    b1_d = nc.dram_tensor("b1", [128, 2], F32, kind="ExternalInput").ap()
    wo1t_d = nc.dram_tensor("wo1t", [128, 2 * O1],
                            FP16 if LG_FP16 else F32R,
                            kind="ExternalInput").ap()
    bo1n_d = nc.dram_tensor("bo1n", [32, O1], F32, kind="ExternalInput").ap()
    w2xt_d = nc.dram_tensor("w2xt", [IN, H], FP16, kind="ExternalInput").ap()
    w2yt_d = nc.dram_tensor("w2yt", [O1, H], FP16, kind="ExternalInput").ap()
    wo2t_d = nc.dram_tensor("wo2t", [128, 2 * O2], FP16, kind="ExternalInput").ap()
    bo2_d = nc.dram_tensor("bo2", [O2, 1], F32, kind="ExternalInput").ap()
    outT1 = nc.dram_tensor("outT1", [O1, rows], FP16, kind="ExternalOutput").ap()
    outT2 = nc.dram_tensor("outT2", [O2, rows], FP16, kind="ExternalOutput").ap()

    with tile.TileContext(nc) as tc:
        with tc.tile_pool(name="const", bufs=1) as cpool, \
             tc.tile_pool(name="xin", bufs=5) as xin, \
             tc.tile_pool(name="h1sb", bufs=6) as h1sb, \
             tc.tile_pool(name="nat", bufs=3) as nat, \
             tc.tile_pool(name="tt", bufs=5) as tt, \
             tc.tile_pool(name="h2sb", bufs=8) as h2sb, \
             tc.tile_pool(name="h1ps", bufs=2, space="PSUM") as h1ps, \
             tc.tile_pool(name="lgps", bufs=1, space="PSUM") as lgps, \
             tc.tile_pool(name="h2ps", bufs=2, space="PSUM") as h2ps, \
             tc.tile_pool(name="y2ps", bufs=1, space="PSUM") as y2ps:

            # --- constants into SBUF ---
            w1t_hi_sb = cpool.tile_from(w1t_hi)
            w1t_lo_sb = cpool.tile_from(w1t_lo)
            b1_sb = cpool.tile_from(b1_d)
            wo1t_sb = cpool.tile_from(wo1t_d)
            bo1n_sb = cpool.tile_from(bo1n_d)
            w2xt_sb = cpool.tile_from(w2xt_d)
            w2yt_sb = cpool.tile_from(w2yt_d)
            wo2t_sb = cpool.tile_from(wo2t_d)
            bo2_sb = cpool.tile_from(bo2_d)

            # Software pipeline over macros; argmax/y2/DMA work at pair
            # (2-macro) granularity. Stage order within an iteration: input
            # DMA + h1 (m), logits+argmax (pair ending at m-1), y2+sigmoid+
            # out-DMAs (pair ending at m-5), h2 (m-3). The PE never waits on
            # the DVE argmax chain or the relu/sigmoid ops.
            S = {}     # per-macro state
            P = {}     # per-pair state, keyed by pair index
            XQ = []    # prefetched X pair tiles
            for m in range(n_macros + 9):
                if m < n_macros:
                    c0 = m * MACRO
                    if m == 0:
                        XQ.append(xin.tile([IN, PAIR], FP16, tag="xp", name="xp0"))
                        nc.sync.dma_start(XQ[-1][:], xt[:, 0:PAIR])
                    if m % 2 == 0 and m + 2 < n_macros:
                        XQ.append(xin.tile([IN, PAIR], FP16, tag="xp", name="xpn"))
                        nc.sync.dma_start(
                            XQ[-1][:], xt[:, c0 + PAIR:c0 + 2 * PAIR])
                    xp = XQ[m // 2]
                    xh = xp[:, (m % 2) * MACRO:(m % 2 + 1) * MACRO]

                    # --- stage 1: h1T = relu(W1 @ X.T + b1) ---
                    h1t = []
                    for c in range(2):
                        ps = h1ps.tile([128, MACRO], F32, tag="h1ps")
                        wh = w1t_hi_sb[:, 128 * c:128 * (c + 1)]
                        wl = w1t_lo_sb[:, 128 * c:128 * (c + 1)]
                        if STAGE1_TERMS == 1:
                            nc.tensor.matmul(ps[:], wh, xh, start=True, stop=True)
                        else:
                            nc.tensor.matmul(ps[:], wh, xh, start=True, stop=False)
                            nc.tensor.matmul(ps[:], wl, xh, start=False, stop=True)
                        sb = h1sb.tile([128, MACRO],
                                       FP16 if LG_FP16 else F32R, tag="h1")
                        if c == 0:
                            # relu on DVE: (psum + b1) max 0
                            nc.vector.tensor_scalar(
                                sb[:], ps[:], b1_sb[:, c:c + 1], 0.0,
                                mybir.AluOpType.add, mybir.AluOpType.max)
                        else:
                            # relu on ACT (engine balance)
                            nc.scalar.activation(
                                sb[:], ps[:], mybir.ActivationFunctionType.Relu,
                                bias=b1_sb[:, c:c + 1], scale=1.0)
                        h1t.append(sb)
                    S[m] = {"c0": c0, "xh": xh, "h1t": h1t}

                # --- logits + argmax for pair (m-2, m-1), at odd m-1 ---
                if m >= 2 and m % 2 == 0 and m - 2 in S:
                    pidx = (m - 2) // 2
                    lg = lgps.tile([32, PAIR], F32, tag="lg", name="lg")
                    for k in range(2):           # macro m-2+k
                        st = S[m - 2 + k]
                        for c in range(2):
                            nc.tensor.matmul(
                                lg[0:O1, MACRO * k:MACRO * (k + 1)],
                                wo1t_sb[:, O1 * c:O1 * (c + 1)],
                                st["h1t"][c][:],
                                start=(c == 0), stop=(c == 1))

                    # block-transpose: lgn[i, 32b + j] = logit_j(row 32b + i)
                    lgn = nat.tile([32, PAIR], F32, tag="lgn")
                    nc.vector.transpose(lgn[:], lg[:])
                    lgn3 = lgn[:].rearrange("p (b j) -> p b j", j=32)[:, :, 0:O1]
                    if bo1_nonzero:
                        bo1_b = bo1n_sb[:].unsqueeze(1).broadcast_to(
                            [32, PAIR // 32, O1])
                        nc.vector.tensor_tensor(
                            lgn3, lgn3, bo1_b, mybir.AluOpType.add)

                    # argmax -> one-hot (exact f32 compare)
                    mx = nat.tile([32, PAIR // 32], F32, tag="mx")
                    nc.vector.tensor_reduce(
                        out=mx[:], in_=lgn3, op=mybir.AluOpType.max,
                        axis=mybir.AxisListType.X)
                    oh = nat.tile([32, PAIR], FP16, tag="oh")
                    oh3 = oh[:].rearrange("p (b j) -> p b j", j=32)[:, :, 0:O1]
                    mx_b = mx[:].unsqueeze(2).broadcast_to([32, PAIR // 32, O1])
                    nc.vector.tensor_tensor(oh3, lgn3, mx_b, mybir.AluOpType.is_equal)

                    # block-transpose back: t[j, r] = one_hot, j in 0..9
                    t_out = tt.tile([32, PAIR], FP16, tag="t_out")
                    nc.vector.transpose(t_out[:], oh[:])
                    P[pidx] = {"t": t_out, "c0": S[m - 2]["c0"]}

                # --- y2 + sigmoid + output DMAs for pair (m-7, m-6) ---
                if m >= 7 and m % 2 == 1 and (m - 7) // 2 in P:
                    pidx = (m - 7) // 2
                    pst = P[pidx]
                    y2p = y2ps.tile([O2, PAIR], F32, tag="y2ps")
                    for k in range(2):           # macro m-7+k
                        h2t = S[m - 7 + k]["h2t"]
                        for c in range(2):
                            nc.tensor.matmul(
                                y2p[:, MACRO * k:MACRO * (k + 1)],
                                wo2t_sb[:, O2 * c:O2 * (c + 1)],
                                h2t[c][:], start=(c == 0), stop=(c == 1))
                    y2t = tt.tile([O2, PAIR], FP16, tag="y2t")
                    nc.scalar.activation(
                        y2t[:], y2p[:], mybir.ActivationFunctionType.Sigmoid,
                        bias=bo2_sb[:, 0:1], scale=1.0)

                    pc0 = pst["c0"]
                    nc.sync.dma_start(outT1[:, pc0:pc0 + PAIR], pst["t"][0:O1, :])
                    nc.sync.dma_start(outT2[:, pc0:pc0 + PAIR], y2t[:])
                    del P[pidx]
                    del S[m - 7]
                    del S[m - 6]

                # --- stage 2 for macro m-5: h2T = relu(W2x@X.T + W2y'@y1T) ---
                if m >= 5 and m - 5 in S:
                    st = S[m - 5]
                    y1t = P[(m - 5) // 2]["t"]
                    off = ((m - 5) % 2) * MACRO
                    pss = []
                    for c in range(2):
                        ps = h2ps.tile([128, MACRO], F32, tag="h2ps")
                        nc.tensor.matmul(
                            ps[:], w2xt_sb[:, 128 * c:128 * (c + 1)], st["xh"],
                            start=True, stop=False)
                        pss.append(ps)
                    h2t = []
                    for c in range(2):
                        ps = pss[c]
                        nc.tensor.matmul(
                            ps[:], w2yt_sb[:, 128 * c:128 * (c + 1)],
                            y1t[0:O1, off:off + MACRO], start=False, stop=True)
                        sb = h2sb.tile([128, MACRO], FP16, tag="h2")
                        nc.scalar.activation(
                            sb[:], ps[:], mybir.ActivationFunctionType.Relu,
                            bias=0.0, scale=1.0)
                        h2t.append(sb)
                    st["h2t"] = h2t
    nc.compile()
    return nc


def _prep_inputs(X, W1, b1, Wo1, bo1, W2, b2, Wo2, bo2, rows_per_core, n_cores):
    """Host-side prep: shard + transpose X to fp16, split W1 hi/lo, pack."""
    X = np.asarray(X, dtype=np.float32)
    W1 = np.asarray(W1, dtype=np.float32)
    b1 = np.asarray(b1, dtype=np.float32)
    Wo1 = np.asarray(Wo1, dtype=np.float32)
    bo1 = np.asarray(bo1, dtype=np.float32)
    W2 = np.asarray(W2, dtype=np.float32)
    b2 = np.asarray(b2, dtype=np.float32)
    Wo2 = np.asarray(Wo2, dtype=np.float32)
    bo2 = np.asarray(bo2, dtype=np.float32)

    s1 = S1 if LG_FP16 else 1.0
    so = SO if LG_FP16 else 1.0
    w1t = np.ascontiguousarray(W1.T) * s1                # [128, 256]
    w1t_hi = w1t.astype(np.float16)
    w1t_lo = (w1t - w1t_hi.astype(np.float32)).astype(np.float16)
    w2t = W2.T                                           # [138, 256]
    w2xt = np.ascontiguousarray(w2t[:IN]).astype(np.float16)
    # fold b2 into the one-hot weight columns: y1 @ (W2y + b2 1^T).T = y1@W2y.T + b2
    w2yt = np.ascontiguousarray(w2t[IN:] + b2[None, :]).astype(np.float16)
    wo1t = np.ascontiguousarray(Wo1.T) * so              # [256, 10]
    wo1t_p = np.concatenate([wo1t[:128], wo1t[128:]], axis=1)  # [128, 20]
    if LG_FP16:
        wo1t_p = wo1t_p.astype(np.float16)
    wo2t = np.ascontiguousarray(Wo2.T).astype(np.float16)      # [256, 6]
    wo2t_p = np.concatenate([wo2t[:128], wo2t[128:]], axis=1)  # [128, 12]

    common = {
        "w1t_hi": w1t_hi, "w1t_lo": w1t_lo,
        "b1": np.ascontiguousarray((b1 * s1).reshape(2, 128).T),
        "wo1t": wo1t_p,
        "bo1n": np.ascontiguousarray(np.broadcast_to(bo1 * (s1 * so), (32, O1))),
        "w2xt": w2xt, "w2yt": w2yt,
        "wo2t": wo2t_p,
        "bo2": np.ascontiguousarray(bo2.reshape(O2, 1)),
    }

    in_maps = []
    for c in range(n_cores):
        Xs = X[c * rows_per_core:(c + 1) * rows_per_core]
        xt = np.ascontiguousarray(Xs.T).astype(np.float16)   # [128, rows]
        in_maps.append({**common, "xt": xt})
    return in_maps


_NC_CACHE = {}


def _get_nc(n_macros, bo1_nonzero):
    key = (n_macros, bo1_nonzero)
    if key not in _NC_CACHE:
        _NC_CACHE[key] = build(n_macros, bo1_nonzero)
    return _NC_CACHE[key]


def run(X, W1, b1, Wo1, bo1, W2, b2, Wo2, bo2, trace=False):
    """Full-size run across 8 cores. Returns (out [B,16] f32, exec_time_ns|None)."""
    n_macros = SHARD // MACRO
    bo1_nonzero = bool(np.any(np.asarray(bo1)))
    nc = _get_nc(n_macros, bo1_nonzero)
    in_maps = _prep_inputs(X, W1, b1, Wo1, bo1, W2, b2, Wo2, bo2, SHARD, N_CORES)
    res = run_bass_kernel_spmd(nc, in_maps, core_ids=list(range(N_CORES)), trace=trace)
    out = np.empty((BATCH, OUT), dtype=np.float32)
    for c in range(N_CORES):
        r = res.results[c]
        out[c * SHARD:(c + 1) * SHARD, :O1] = r["outT1"].T.astype(np.float32)
        out[c * SHARD:(c + 1) * SHARD, O1:] = r["outT2"].T.astype(np.float32)
    return out, res.exec_time_ns


def kernel(X, W1, b1, Wo1, bo1, W2, b2, Wo2, bo2):
    out, _ = run(X, W1, b1, Wo1, bo1, W2, b2, Wo2, bo2)
    return out


# revision 20
# speedup vs baseline: 1.0229x; 1.0229x over previous
"""Trainium2 Bass kernel for nn_AnimationPredictor (2-layer MLP with argmax/one-hot).

Data-parallel over 8 NeuronCores: each core processes 65536 rows.

Math per row (reference):
  h1 = relu(X @ W1.T + b1)            [B, 256]
  logits = h1 @ Wo1.T + bo1           [B, 10]
  y1 = one_hot(argmax(logits), 10)
  h2 = relu(concat([X, y1]) @ W2.T + b2)   [B, 256]
  y2 = sigmoid(h2 @ Wo2.T + bo2)      [B, 6]
  out = concat([y1, y2])              [B, 16]

On-chip layout: batch rows on the matmul free dim ("T layout"). X arrives
pre-transposed as fp16 and stage 1 runs as a single fp16 matmul per
128-feature half (STAGE1_TERMS=1; the fp16 input/weight quantization
flips argmax on ~3e-4 of rows, rel l2 1.61e-2 vs the 2e-2 gate,
deterministic for the fixed reference inputs; STAGE1_TERMS=2 adds a W1-lo
correction term at 1.47e-2). Logits are computed in T layout with f32r
operands (f32 exponent, ~11-bit mantissa - preserves tiny relu outputs
that fp16 would denormalize). The argmax machinery runs at two-macro
granularity (1024 rows) to amortize fixed per-op costs: a 32x32-block
STREAM_TRANSPOSE turns the [32, 1024] logits psum into an interleaved
natural layout where the 10 classes sit on the free dim, so max/one-hot
are cheap free-dim DVE ops; a second block transpose returns the one-hot
to T layout for stage 2. b2 is folded into W2y on the host (one-hot rows
sum to exactly 1), so the h2 relus need no bias. y2 + sigmoid + all DMAs
also run at two-macro granularity. bo1 is applied with a natural-layout
add only when nonzero (the reference uses zero biases). The loop is
software-pipelined 8 deep (h1 at lag 0, logits/argmax at lag 1-2, h2 at
lag 4, y2/output at lag 6-7) so the PE never waits on the DVE argmax
chain; relus split across DVE and ACT to balance engines. Outputs leave
feature-major fp16; the host transposes and casts.
"""
import sys

sys.path.insert(0, "/opt/trn_rl_repo")

import numpy as np

import concourse.bass as bass
import concourse.tile as tile
from concourse import bacc, mybir
from concourse.bass_utils import run_bass_kernel_spmd

F32 = mybir.dt.float32
F32R = mybir.dt.float32r
FP16 = mybir.dt.float16

N_CORES = 8
BATCH = 524288
IN = 128
H = 256
O1 = 10
O2 = 6
OUT = O1 + O2
SHARD = BATCH // N_CORES          # 65536 rows per core
MACRO = 512                       # rows per macro-tile (one PSUM bank at f32)
PAIR = 2 * MACRO                  # argmax/y2/DMA granularity
# fp16 logits path: fold x256 into W1/b1 (relu commutes with positive
# scale; h1 only feeds the logits) and x16 into Wo1, so the logits matmul
# runs in plain fp16 (1 cyc/row, small LDW) with no subnormal loss.
# Costs ~40 extra argmax flips vs f32r (sim: rel 1.69e-2 vs 1.47e-2).
LG_FP16 = True
S1 = 256.0
SO = 16.0


def build(n_macros=SHARD // MACRO, bo1_nonzero=False):
    assert n_macros % 2 == 0
    nc = bacc.Bacc("TRN2", target_bir_lowering=False, debug=False)
    rows = n_macros * MACRO

    # --- DRAM parameters (per-core shapes) ---
    xt = nc.dram_tensor("xt", [IN, rows], FP16, kind="ExternalInput").ap()
    w1t_hi = nc.dram_tensor("w1t_hi", [IN, H], FP16, kind="ExternalInput").ap()
    w1t_lo = nc.dram_tensor("w1t_lo", [IN, H], FP16, kind="ExternalInput").ap()
    b1_d = nc.dram_tensor("b1", [128, 2], F32, kind="ExternalInput").ap()
    wo1t_d = nc.dram_tensor("wo1t", [128, 2 * O1],
                            FP16 if LG_FP16 else F32R,
                            kind="ExternalInput").ap()
    bo1n_d = nc.dram_tensor("bo1n", [32, O1], F32, kind="ExternalInput").ap()
    w2xt_d = nc.dram_tensor("w2xt", [IN, H], FP16, kind="ExternalInput").ap()
    w2yt_d = nc.dram_tensor("w2yt", [O1, H], FP16, kind="ExternalInput").ap()
    wo2t_d = nc.dram_tensor("wo2t", [128, 2 * O2], FP16, kind="ExternalInput").ap()
    bo2_d = nc.dram_tensor("bo2", [O2, 1], F32, kind="ExternalInput").ap()
    outT1 = nc.dram_tensor("outT1", [O1, rows], FP16, kind="ExternalOutput").ap()
    outT2 = nc.dram_tensor("outT2", [O2, rows], FP16, kind="ExternalOutput").ap()

    with tile.TileContext(nc) as tc:
        with tc.tile_pool(name="const", bufs=1) as cpool, \
             tc.tile_pool(name="xin", bufs=5) as xin, \
             tc.tile_pool(name="h1sb", bufs=6) as h1sb, \
             tc.tile_pool(name="nat", bufs=3) as nat, \
             tc.tile_pool(name="tt", bufs=5) as tt, \
             tc.tile_pool(name="h2sb", bufs=8) as h2sb, \
             tc.tile_pool(name="h1ps", bufs=2, space="PSUM") as h1ps, \
             tc.tile_pool(name="lgps", bufs=1, space="PSUM") as lgps, \
             tc.tile_pool(name="h2ps", bufs=2, space="PSUM") as h2ps, \
             tc.tile_pool(name="y2ps", bufs=1, space="PSUM") as y2ps:

            # --- constants into SBUF ---
            w1t_hi_sb = cpool.tile_from(w1t_hi)
            w1t_lo_sb = cpool.tile_from(w1t_lo)
            b1_sb = cpool.tile_from(b1_d)
            wo1t_sb = cpool.tile_from(wo1t_d)
            bo1n_sb = cpool.tile_from(bo1n_d)
            w2xt_sb = cpool.tile_from(w2xt_d)
            w2yt_sb = cpool.tile_from(w2yt_d)
            wo2t_sb = cpool.tile_from(wo2t_d)
            bo2_sb = cpool.tile_from(bo2_d)

            # Software pipeline over macros; argmax/y2/DMA work at pair
            # (2-macro) granularity. Stage order within an iteration: input
            # DMA + h1 (m), logits+argmax (pair ending at m-1), y2+sigmoid+
            # out-DMAs (pair ending at m-5), h2 (m-3). The PE never waits on
            # the DVE argmax chain or the relu/sigmoid ops.
            S = {}     # per-macro state
            P = {}     # per-pair state, keyed by pair index
            XQ = []    # prefetched X pair tiles
            for m in range(n_macros + 8):
                if m < n_macros:
                    c0 = m * MACRO
                    if m == 0:
                        XQ.append(xin.tile([IN, PAIR], FP16, tag="xp", name="xp0"))
                        nc.sync.dma_start(XQ[-1][:], xt[:, 0:PAIR])
                    if m % 2 == 0 and m + 2 < n_macros:
                        XQ.append(xin.tile([IN, PAIR], FP16, tag="xp", name="xpn"))
                        nc.sync.dma_start(
                            XQ[-1][:], xt[:, c0 + PAIR:c0 + 2 * PAIR])
                    xp = XQ[m // 2]
                    xh = xp[:, (m % 2) * MACRO:(m % 2 + 1) * MACRO]

                    # --- stage 1: h1T = relu(W1 @ X.T + b1) ---
                    h1t = []
                    for c in range(2):
                        ps = h1ps.tile([128, MACRO], F32, tag="h1ps")
                        wh = w1t_hi_sb[:, 128 * c:128 * (c + 1)]
                        wl = w1t_lo_sb[:, 128 * c:128 * (c + 1)]
                        if STAGE1_TERMS == 1:
                            nc.tensor.matmul(ps[:], wh, xh, start=True, stop=True)
                        else:
                            nc.tensor.matmul(ps[:], wh, xh, start=True, stop=False)
                            nc.tensor.matmul(ps[:], wl, xh, start=False, stop=True)
                        sb = h1sb.tile([128, MACRO],
                                       FP16 if LG_FP16 else F32R, tag="h1")
                        if c == 0:
                            # relu on DVE: (psum + b1) max 0
                            nc.vector.tensor_scalar(
                                sb[:], ps[:], b1_sb[:, c:c + 1], 0.0,
                                mybir.AluOpType.add, mybir.AluOpType.max)
                        else:
                            # relu on ACT (engine balance)
                            nc.scalar.activation(
                                sb[:], ps[:], mybir.ActivationFunctionType.Relu,
                                bias=b1_sb[:, c:c + 1], scale=1.0)
                        h1t.append(sb)
                    S[m] = {"c0": c0, "xh": xh, "h1t": h1t}

                # --- logits + argmax for pair (m-2, m-1), at odd m-1 ---
                if m >= 2 and m % 2 == 0 and m - 2 in S:
                    pidx = (m - 2) // 2
                    lg = lgps.tile([32, PAIR], F32, tag="lg", name="lg")
                    for k in range(2):           # macro m-2+k
                        st = S[m - 2 + k]
                        for c in range(2):
                            nc.tensor.matmul(
                                lg[0:O1, MACRO * k:MACRO * (k + 1)],
                                wo1t_sb[:, O1 * c:O1 * (c + 1)],
                                st["h1t"][c][:],
                                start=(c == 0), stop=(c == 1))

                    # block-transpose: lgn[i, 32b + j] = logit_j(row 32b + i)
                    lgn = nat.tile([32, PAIR], F32, tag="lgn")
                    nc.vector.transpose(lgn[:], lg[:])
                    lgn3 = lgn[:].rearrange("p (b j) -> p b j", j=32)[:, :, 0:O1]
                    if bo1_nonzero:
                        bo1_b = bo1n_sb[:].unsqueeze(1).broadcast_to(
                            [32, PAIR // 32, O1])
                        nc.vector.tensor_tensor(
                            lgn3, lgn3, bo1_b, mybir.AluOpType.add)

                    # argmax -> one-hot (exact f32 compare)
                    mx = nat.tile([32, PAIR // 32], F32, tag="mx")
                    nc.vector.tensor_reduce(
                        out=mx[:], in_=lgn3, op=mybir.AluOpType.max,
                        axis=mybir.AxisListType.X)
                    oh = nat.tile([32, PAIR], FP16, tag="oh")
                    oh3 = oh[:].rearrange("p (b j) -> p b j", j=32)[:, :, 0:O1]
                    mx_b = mx[:].unsqueeze(2).broadcast_to([32, PAIR // 32, O1])
                    nc.vector.tensor_tensor(oh3, lgn3, mx_b, mybir.AluOpType.is_equal)

                    # block-transpose back: t[j, r] = one_hot, j in 0..9
                    t_out = tt.tile([32, PAIR], FP16, tag="t_out")
                    nc.vector.transpose(t_out[:], oh[:])
                    P[pidx] = {"t": t_out, "c0": S[m - 2]["c0"]}

                # --- y2 + sigmoid + output DMAs for pair (m-7, m-6) ---
                if m >= 7 and m % 2 == 1 and (m - 7) // 2 in P:
                    pidx = (m - 7) // 2
                    pst = P[pidx]
                    y2p = y2ps.tile([O2, PAIR], F32, tag="y2ps")
                    for k in range(2):           # macro m-7+k
                        h2t = S[m - 7 + k]["h2t"]
                        for c in range(2):
                            nc.tensor.matmul(
                                y2p[:, MACRO * k:MACRO * (k + 1)],
                                wo2t_sb[:, O2 * c:O2 * (c + 1)],
                                h2t[c][:], start=(c == 0), stop=(c == 1))
                    y2t = tt.tile([O2, PAIR], FP16, tag="y2t")
                    nc.scalar.activation(
                        y2t[:], y2p[:], mybir.ActivationFunctionType.Sigmoid,
                        bias=bo2_sb[:, 0:1], scale=1.0)

                    pc0 = pst["c0"]
                    nc.sync.dma_start(outT1[:, pc0:pc0 + PAIR], pst["t"][0:O1, :])
                    nc.sync.dma_start(outT2[:, pc0:pc0 + PAIR], y2t[:])
                    del P[pidx]
                    del S[m - 7]
                    del S[m - 6]

                # --- stage 2 for macro m-4: h2T = relu(W2x@X.T + W2y'@y1T) ---
                if m >= 4 and m - 4 in S:
                    st = S[m - 4]
                    y1t = P[(m - 4) // 2]["t"]
                    off = ((m - 4) % 2) * MACRO
                    pss = []
                    for c in range(2):
                        ps = h2ps.tile([128, MACRO], F32, tag="h2ps")
                        nc.tensor.matmul(
                            ps[:], w2xt_sb[:, 128 * c:128 * (c + 1)], st["xh"],
                            start=True, stop=False)
                        pss.append(ps)
                    h2t = []
                    for c in range(2):
                        ps = pss[c]
                        nc.tensor.matmul(
                            ps[:], w2yt_sb[:, 128 * c:128 * (c + 1)],
                            y1t[0:O1, off:off + MACRO], start=False, stop=True)
                        sb = h2sb.tile([128, MACRO], FP16, tag="h2")
                        nc.scalar.activation(
                            sb[:], ps[:], mybir.ActivationFunctionType.Relu,
                            bias=0.0, scale=1.0)
                        h2t.append(sb)
                    st["h2t"] = h2t
    nc.compile()
    return nc


def _prep_inputs(X, W1, b1, Wo1, bo1, W2, b2, Wo2, bo2, rows_per_core, n_cores):
    """Host-side prep: shard + transpose X to fp16, split W1 hi/lo, pack."""
    X = np.asarray(X, dtype=np.float32)
    W1 = np.asarray(W1, dtype=np.float32)
    b1 = np.asarray(b1, dtype=np.float32)
    Wo1 = np.asarray(Wo1, dtype=np.float32)
    bo1 = np.asarray(bo1, dtype=np.float32)
    W2 = np.asarray(W2, dtype=np.float32)
    b2 = np.asarray(b2, dtype=np.float32)
    Wo2 = np.asarray(Wo2, dtype=np.float32)
    bo2 = np.asarray(bo2, dtype=np.float32)

    s1 = S1 if LG_FP16 else 1.0
    so = SO if LG_FP16 else 1.0
    w1t = np.ascontiguousarray(W1.T) * s1                # [128, 256]
    w1t_hi = w1t.astype(np.float16)
    w1t_lo = (w1t - w1t_hi.astype(np.float32)).astype(np.float16)
    w2t = W2.T                                           # [138, 256]
    w2xt = np.ascontiguousarray(w2t[:IN]).astype(np.float16)
    # fold b2 into the one-hot weight columns: y1 @ (W2y + b2 1^T).T = y1@W2y.T + b2
    w2yt = np.ascontiguousarray(w2t[IN:] + b2[None, :]).astype(np.float16)
    wo1t = np.ascontiguousarray(Wo1.T) * so              # [256, 10]
    wo1t_p = np.concatenate([wo1t[:128], wo1t[128:]], axis=1)  # [128, 20]
    if LG_FP16:
        wo1t_p = wo1t_p.astype(np.float16)
    wo2t = np.ascontiguousarray(Wo2.T).astype(np.float16)      # [256, 6]
    wo2t_p = np.concatenate([wo2t[:128], wo2t[128:]], axis=1)  # [128, 12]

    common = {
        "w1t_hi": w1t_hi, "w1t_lo": w1t_lo,
        "b1": np.ascontiguousarray((b1 * s1).reshape(2, 128).T),
        "wo1t": wo1t_p,
        "bo1n": np.ascontiguousarray(np.broadcast_to(bo1 * (s1 * so), (32, O1))),
        "w2xt": w2xt, "w2yt": w2yt,
        "wo2t": wo2t_p,
        "bo2": np.ascontiguousarray(bo2.reshape(O2, 1)),
    }

    in_maps = []
    for c in range(n_cores):
        Xs = X[c * rows_per_core:(c + 1) * rows_per_core]
        xt = np.ascontiguousarray(Xs.T).astype(np.float16)   # [128, rows]
        in_maps.append({**common, "xt": xt})
    return in_maps


_NC_CACHE = {}


def _get_nc(n_macros, bo1_nonzero):
    key = (n_macros, bo1_nonzero)
    if key not in _NC_CACHE:
        _NC_CACHE[key] = build(n_macros, bo1_nonzero)
    return _NC_CACHE[key]


def run(X, W1, b1, Wo1, bo1, W2, b2, Wo2, bo2, trace=False):
    """Full-size run across 8 cores. Returns (out [B,16] f32, exec_time_ns|None)."""
    n_macros = SHARD // MACRO
    bo1_nonzero = bool(np.any(np.asarray(bo1)))
    nc = _get_nc(n_macros, bo1_nonzero)
    in_maps = _prep_inputs(X, W1, b1, Wo1, bo1, W2, b2, Wo2, bo2, SHARD, N_CORES)
    res = run_bass_kernel_spmd(nc, in_maps, core_ids=list(range(N_CORES)), trace=trace)
    out = np.empty((BATCH, OUT), dtype=np.float32)
    for c in range(N_CORES):
        r = res.results[c]
        out[c * SHARD:(c + 1) * SHARD, :O1] = r["outT1"].T.astype(np.float32)
        out[c * SHARD:(c + 1) * SHARD, O1:] = r["outT2"].T.astype(np.float32)
    return out, res.exec_time_ns


def kernel(X, W1, b1, Wo1, bo1, W2, b2, Wo2, bo2):
    out, _ = run(X, W1, b1, Wo1, bo1, W2, b2, Wo2, bo2)
    return out


# revision 21
# speedup vs baseline: 1.2177x; 1.1904x over previous
"""Trainium2 Bass kernel for nn_AnimationPredictor (2-layer MLP with argmax/one-hot).

Data-parallel over 8 NeuronCores: each core processes 65536 rows.

Math per row (reference):
  h1 = relu(X @ W1.T + b1)            [B, 256]
  logits = h1 @ Wo1.T + bo1           [B, 10]
  y1 = one_hot(argmax(logits), 10)
  h2 = relu(concat([X, y1]) @ W2.T + b2)   [B, 256]
  y2 = sigmoid(h2 @ Wo2.T + bo2)      [B, 6]
  out = concat([y1, y2])              [B, 16]

On-chip layout: batch rows on the matmul free dim ("T layout"). X arrives
pre-transposed as fp16 and stage 1 runs as a single fp16 matmul per
128-feature half (STAGE1_TERMS=1; the fp16 input/weight quantization
flips argmax on ~3e-4 of rows, rel l2 1.61e-2 vs the 2e-2 gate,
deterministic for the fixed reference inputs; STAGE1_TERMS=2 adds a W1-lo
correction term at 1.47e-2). Logits are computed in T layout with f32r
operands (f32 exponent, ~11-bit mantissa - preserves tiny relu outputs
that fp16 would denormalize). The argmax machinery runs at two-macro
granularity (1024 rows) to amortize fixed per-op costs: a 32x32-block
STREAM_TRANSPOSE turns the [32, 1024] logits psum into an interleaved
natural layout where the 10 classes sit on the free dim, so max/one-hot
are cheap free-dim DVE ops; a second block transpose returns the one-hot
to T layout for stage 2. b2 is folded into W2y on the host (one-hot rows
sum to exactly 1), so the h2 relus need no bias. y2 + sigmoid + all DMAs
also run at two-macro granularity. bo1 is applied with a natural-layout
add only when nonzero (the reference uses zero biases). The loop is
software-pipelined 8 deep (h1 at lag 0, logits/argmax at lag 1-2, h2 at
lag 4, y2/output at lag 6-7) so the PE never waits on the DVE argmax
chain; relus split across DVE and ACT to balance engines. Outputs leave
feature-major fp16; the host transposes and casts.
"""
import sys

sys.path.insert(0, "/opt/trn_rl_repo")

import numpy as np

import concourse.bass as bass
import concourse.tile as tile
from concourse import bacc, mybir
from concourse.bass_utils import run_bass_kernel_spmd

F32 = mybir.dt.float32
F32R = mybir.dt.float32r
FP16 = mybir.dt.float16

N_CORES = 8
BATCH = 524288
IN = 128
H = 256
O1 = 10
O2 = 6
OUT = O1 + O2
SHARD = BATCH // N_CORES          # 65536 rows per core
MACRO = 512                       # rows per macro-tile (one PSUM bank at f32)
PAIR = 2 * MACRO                  # argmax/y2/DMA granularity
# fp16 logits path: fold x256 into W1/b1 (relu commutes with positive
# scale; h1 only feeds the logits) and x16 into Wo1, so the logits matmul
# runs in plain fp16 (1 cyc/row, small LDW) with no subnormal loss.
# Costs ~40 extra argmax flips vs f32r (sim: rel 1.69e-2 vs 1.47e-2).
LG_FP16 = True
S1 = 256.0
SO = 16.0


def build(n_macros=SHARD // MACRO, bo1_nonzero=False):
    assert n_macros % 2 == 0
    nc = bacc.Bacc("TRN2", target_bir_lowering=False, debug=False)
    rows = n_macros * MACRO

    # --- DRAM parameters (per-core shapes) ---
    xt = nc.dram_tensor("xt", [IN, rows], FP16, kind="ExternalInput").ap()
    w1t_hi = nc.dram_tensor("w1t_hi", [IN, H], FP16, kind="ExternalInput").ap()
    w1t_lo = nc.dram_tensor("w1t_lo", [IN, H], FP16, kind="ExternalInput").ap()
    b1_d = nc.dram_tensor("b1", [128, 2], F32, kind="ExternalInput").ap()
    wo1t_d = nc.dram_tensor("wo1t", [128, 2 * O1],
                            FP16 if LG_FP16 else F32R,
                            kind="ExternalInput").ap()
    bo1n_d = nc.dram_tensor("bo1n", [32, O1], F32, kind="ExternalInput").ap()
    w2xt_d = nc.dram_tensor("w2xt", [IN, H], FP16, kind="ExternalInput").ap()
    w2yt_d = nc.dram_tensor("w2yt", [O1, H], FP16, kind="ExternalInput").ap()
    wo2t_d = nc.dram_tensor("wo2t", [128, 2 * O2], FP16, kind="ExternalInput").ap()
    bo2_d = nc.dram_tensor("bo2", [O2, 1], F32, kind="ExternalInput").ap()
    outT1 = nc.dram_tensor("outT1", [O1, rows], FP16, kind="ExternalOutput").ap()
    outT2 = nc.dram_tensor("outT2", [O2, rows], FP16, kind="ExternalOutput").ap()

    with tile.TileContext(nc) as tc:
        with tc.tile_pool(name="const", bufs=1) as cpool, \
             tc.tile_pool(name="xin", bufs=5) as xin, \
             tc.tile_pool(name="h1sb", bufs=6) as h1sb, \
             tc.tile_pool(name="nat", bufs=3) as nat, \
             tc.tile_pool(name="tt", bufs=5) as tt, \
             tc.tile_pool(name="h2sb", bufs=8) as h2sb, \
             tc.tile_pool(name="h1ps", bufs=2, space="PSUM") as h1ps, \
             tc.tile_pool(name="lgps", bufs=1, space="PSUM") as lgps, \
             tc.tile_pool(name="h2ps", bufs=2, space="PSUM") as h2ps, \
             tc.tile_pool(name="y2ps", bufs=1, space="PSUM") as y2ps:

            # --- constants into SBUF ---
            w1t_hi_sb = cpool.tile_from(w1t_hi)
            w1t_lo_sb = cpool.tile_from(w1t_lo)
            b1_sb = cpool.tile_from(b1_d)
            wo1t_sb = cpool.tile_from(wo1t_d)
            bo1n_sb = cpool.tile_from(bo1n_d)
            w2xt_sb = cpool.tile_from(w2xt_d)
            w2yt_sb = cpool.tile_from(w2yt_d)
            wo2t_sb = cpool.tile_from(wo2t_d)
            bo2_sb = cpool.tile_from(bo2_d)

            # Software pipeline over macros; argmax/y2/DMA work at pair
            # (2-macro) granularity. Stage order within an iteration: input
            # DMA + h1 (m), logits+argmax (pair ending at m-1), y2+sigmoid+
            # out-DMAs (pair ending at m-5), h2 (m-3). The PE never waits on
            # the DVE argmax chain or the relu/sigmoid ops.
            S = {}     # per-macro state
            P = {}     # per-pair state, keyed by pair index
            XQ = []    # prefetched X pair tiles
            for m in range(n_macros + 8):
                # stage-2 MMs for macro m-4, prepared up front so they can
                # interleave with the logits/y2 bursts (independent psum
                # banks; their deps are several iterations old)
                h2q = []
                if m >= 4 and m - 4 in S:
                    stC = S[m - 4]
                    y1tC = P[(m - 4) // 2]["t"]
                    offC = ((m - 4) % 2) * MACRO
                    pssC = [h2ps.tile([128, MACRO], F32, tag="h2ps", name="h2a"),
                            h2ps.tile([128, MACRO], F32, tag="h2ps", name="h2b")]
                    for c in range(2):
                        h2q.append((pssC[c], w2xt_sb[:, 128 * c:128 * (c + 1)],
                                    stC["xh"], True, False))
                    for c in range(2):
                        h2q.append((pssC[c], w2yt_sb[:, 128 * c:128 * (c + 1)],
                                    y1tC[0:O1, offC:offC + MACRO], False, True))

                def emit_h2():
                    if h2q:
                        ps_, lhs_, rhs_, st_, sp_ = h2q.pop(0)
                        nc.tensor.matmul(ps_[:], lhs_, rhs_, start=st_, stop=sp_)
                if m < n_macros:
                    c0 = m * MACRO
                    if m == 0:
                        XQ.append(xin.tile([IN, PAIR], FP16, tag="xp", name="xp0"))
                        nc.sync.dma_start(XQ[-1][:], xt[:, 0:PAIR])
                    if m % 2 == 0 and m + 2 < n_macros:
                        XQ.append(xin.tile([IN, PAIR], FP16, tag="xp", name="xpn"))
                        nc.sync.dma_start(
                            XQ[-1][:], xt[:, c0 + PAIR:c0 + 2 * PAIR])
                    xp = XQ[m // 2]
                    xh = xp[:, (m % 2) * MACRO:(m % 2 + 1) * MACRO]

                    # --- stage 1: h1T = relu(W1 @ X.T + b1) ---
                    h1t = []
                    for c in range(2):
                        ps = h1ps.tile([128, MACRO], F32, tag="h1ps")
                        wh = w1t_hi_sb[:, 128 * c:128 * (c + 1)]
                        wl = w1t_lo_sb[:, 128 * c:128 * (c + 1)]
                        if STAGE1_TERMS == 1:
                            nc.tensor.matmul(ps[:], wh, xh, start=True, stop=True)
                        else:
                            nc.tensor.matmul(ps[:], wh, xh, start=True, stop=False)
                            nc.tensor.matmul(ps[:], wl, xh, start=False, stop=True)
                        sb = h1sb.tile([128, MACRO],
                                       FP16 if LG_FP16 else F32R, tag="h1")
                        if c == 0:
                            # relu on DVE: (psum + b1) max 0
                            nc.vector.tensor_scalar(
                                sb[:], ps[:], b1_sb[:, c:c + 1], 0.0,
                                mybir.AluOpType.add, mybir.AluOpType.max)
                        else:
                            # relu on ACT (engine balance)
                            nc.scalar.activation(
                                sb[:], ps[:], mybir.ActivationFunctionType.Relu,
                                bias=b1_sb[:, c:c + 1], scale=1.0)
                        h1t.append(sb)
                    S[m] = {"c0": c0, "xh": xh, "h1t": h1t}

                # --- logits + argmax for pair (m-2, m-1), at odd m-1 ---
                if m >= 2 and m % 2 == 0 and m - 2 in S:
                    pidx = (m - 2) // 2
                    lg = lgps.tile([32, PAIR], F32, tag="lg", name="lg")
                    for k in range(2):           # macro m-2+k
                        st = S[m - 2 + k]
                        for c in range(2):
                            nc.tensor.matmul(
                                lg[0:O1, MACRO * k:MACRO * (k + 1)],
                                wo1t_sb[:, O1 * c:O1 * (c + 1)],
                                st["h1t"][c][:],
                                start=(c == 0), stop=(c == 1))
                            emit_h2()

                    # block-transpose: lgn[i, 32b + j] = logit_j(row 32b + i)
                    lgn = nat.tile([32, PAIR], F32, tag="lgn")
                    nc.vector.transpose(lgn[:], lg[:])
                    lgn3 = lgn[:].rearrange("p (b j) -> p b j", j=32)[:, :, 0:O1]
                    if bo1_nonzero:
                        bo1_b = bo1n_sb[:].unsqueeze(1).broadcast_to(
                            [32, PAIR // 32, O1])
                        nc.vector.tensor_tensor(
                            lgn3, lgn3, bo1_b, mybir.AluOpType.add)

                    # argmax -> one-hot (exact f32 compare)
                    mx = nat.tile([32, PAIR // 32], F32, tag="mx")
                    nc.vector.tensor_reduce(
                        out=mx[:], in_=lgn3, op=mybir.AluOpType.max,
                        axis=mybir.AxisListType.X)
                    oh = nat.tile([32, PAIR], FP16, tag="oh")
                    oh3 = oh[:].rearrange("p (b j) -> p b j", j=32)[:, :, 0:O1]
                    mx_b = mx[:].unsqueeze(2).broadcast_to([32, PAIR // 32, O1])
                    nc.vector.tensor_tensor(oh3, lgn3, mx_b, mybir.AluOpType.is_equal)

                    # block-transpose back: t[j, r] = one_hot, j in 0..9
                    t_out = tt.tile([32, PAIR], FP16, tag="t_out")
                    nc.vector.transpose(t_out[:], oh[:])
                    P[pidx] = {"t": t_out, "c0": S[m - 2]["c0"]}

                # --- y2 + sigmoid + output DMAs for pair (m-7, m-6) ---
                if m >= 7 and m % 2 == 1 and (m - 7) // 2 in P:
                    pidx = (m - 7) // 2
                    pst = P[pidx]
                    y2p = y2ps.tile([O2, PAIR], F32, tag="y2ps")
                    for k in range(2):           # macro m-7+k
                        h2t = S[m - 7 + k]["h2t"]
                        for c in range(2):
                            nc.tensor.matmul(
                                y2p[:, MACRO * k:MACRO * (k + 1)],
                                wo2t_sb[:, O2 * c:O2 * (c + 1)],
                                h2t[c][:], start=(c == 0), stop=(c == 1))
                            emit_h2()
                    y2t = tt.tile([O2, PAIR], FP16, tag="y2t")
                    nc.scalar.activation(
                        y2t[:], y2p[:], mybir.ActivationFunctionType.Sigmoid,
                        bias=bo2_sb[:, 0:1], scale=1.0)

                    pc0 = pst["c0"]
                    nc.sync.dma_start(outT1[:, pc0:pc0 + PAIR], pst["t"][0:O1, :])
                    nc.sync.dma_start(outT2[:, pc0:pc0 + PAIR], y2t[:])
                    del P[pidx]
                    del S[m - 7]
                    del S[m - 6]

                # --- stage 2 for macro m-4: h2T = relu(W2x@X.T + W2y'@y1T) ---
                if m >= 4 and m - 4 in S:
                    while h2q:
                        emit_h2()
                    h2t = []
                    for c in range(2):
                        sb = h2sb.tile([128, MACRO], FP16, tag="h2")
                        nc.scalar.activation(
                            sb[:], pssC[c][:], mybir.ActivationFunctionType.Relu,
                            bias=0.0, scale=1.0)
                        h2t.append(sb)
                    stC["h2t"] = h2t
    nc.compile()
    return nc


def _prep_inputs(X, W1, b1, Wo1, bo1, W2, b2, Wo2, bo2, rows_per_core, n_cores):
    """Host-side prep: shard + transpose X to fp16, split W1 hi/lo, pack."""
    X = np.asarray(X, dtype=np.float32)
    W1 = np.asarray(W1, dtype=np.float32)
    b1 = np.asarray(b1, dtype=np.float32)
    Wo1 = np.asarray(Wo1, dtype=np.float32)
    bo1 = np.asarray(bo1, dtype=np.float32)
    W2 = np.asarray(W2, dtype=np.float32)
    b2 = np.asarray(b2, dtype=np.float32)
    Wo2 = np.asarray(Wo2, dtype=np.float32)
    bo2 = np.asarray(bo2, dtype=np.float32)

    s1 = S1 if LG_FP16 else 1.0
    so = SO if LG_FP16 else 1.0
    w1t = np.ascontiguousarray(W1.T) * s1                # [128, 256]
    w1t_hi = w1t.astype(np.float16)
    w1t_lo = (w1t - w1t_hi.astype(np.float32)).astype(np.float16)
    w2t = W2.T                                           # [138, 256]
    w2xt = np.ascontiguousarray(w2t[:IN]).astype(np.float16)
    # fold b2 into the one-hot weight columns: y1 @ (W2y + b2 1^T).T = y1@W2y.T + b2
    w2yt = np.ascontiguousarray(w2t[IN:] + b2[None, :]).astype(np.float16)
    wo1t = np.ascontiguousarray(Wo1.T) * so              # [256, 10]
    wo1t_p = np.concatenate([wo1t[:128], wo1t[128:]], axis=1)  # [128, 20]
    if LG_FP16:
        wo1t_p = wo1t_p.astype(np.float16)
    wo2t = np.ascontiguousarray(Wo2.T).astype(np.float16)      # [256, 6]
    wo2t_p = np.concatenate([wo2t[:128], wo2t[128:]], axis=1)  # [128, 12]

    common = {
        "w1t_hi": w1t_hi, "w1t_lo": w1t_lo,
        "b1": np.ascontiguousarray((b1 * s1).reshape(2, 128).T),
        "wo1t": wo1t_p,
        "bo1n": np.ascontiguousarray(np.broadcast_to(bo1 * (s1 * so), (32, O1))),
        "w2xt": w2xt, "w2yt": w2yt,
        "wo2t": wo2t_p,
        "bo2": np.ascontiguousarray(bo2.reshape(O2, 1)),
    }

    in_maps = []
    for c in range(n_cores):
        Xs = X[c * rows_per_core:(c + 1) * rows_per_core]
        xt = np.ascontiguousarray(Xs.T).astype(np.float16)   # [128, rows]
        in_maps.append({**common, "xt": xt})
    return in_maps


_NC_CACHE = {}


def _get_nc(n_macros, bo1_nonzero):
    key = (n_macros, bo1_nonzero)
    if key not in _NC_CACHE:
        _NC_CACHE[key] = build(n_macros, bo1_nonzero)
    return _NC_CACHE[key]


def run(X, W1, b1, Wo1, bo1, W2, b2, Wo2, bo2, trace=False):
    """Full-size run across 8 cores. Returns (out [B,16] f32, exec_time_ns|None)."""
    n_macros = SHARD // MACRO
    bo1_nonzero = bool(np.any(np.asarray(bo1)))
    nc = _get_nc(n_macros, bo1_nonzero)
    in_maps = _prep_inputs(X, W1, b1, Wo1, bo1, W2, b2, Wo2, bo2, SHARD, N_CORES)
    res = run_bass_kernel_spmd(nc, in_maps, core_ids=list(range(N_CORES)), trace=trace)
    out = np.empty((BATCH, OUT), dtype=np.float32)
    for c in range(N_CORES):
        r = res.results[c]
        out[c * SHARD:(c + 1) * SHARD, :O1] = r["outT1"].T.astype(np.float32)
        out[c * SHARD:(c + 1) * SHARD, O1:] = r["outT2"].T.astype(np.float32)
    return out, res.exec_time_ns


def kernel(X, W1, b1, Wo1, bo1, W2, b2, Wo2, bo2):
    out, _ = run(X, W1, b1, Wo1, bo1, W2, b2, Wo2, bo2)
    return out
